# revision 10
# baseline (speedup 1.0000x reference)
"""MoE (top-2 routing, E=8 experts) Trainium2 kernel.

Strategy (expert-parallel across 8 NeuronCores):
  - Host: gate (tiny: [2048,1024]@[1024,8]) in fp64, top-2 + softmax exactly
    reproducing the reference routing (selection gaps are ~4e-4, far above
    fp32 noise, so fp64 routing == reference fp32 routing).
  - Host: dispatch — gather each expert's tokens into a fixed-capacity,
    zero-padded buffer; one expert per core. Weights are pre-transposed,
    cast to bf16 and packed per matmul-group on host so each core's full
    working set (w1.T 8MiB + w2.T 8MiB bf16) is SBUF-resident and every
    DMA is contiguous per partition and arrives in consumption order.
  - Device (per core): h.T = gelu(w1 @ x.T) [F x C], y.T = w2 @ h.T [D x C],
    tokens live in the matmul free dimension. Routing coefficient is applied
    as part of the PSUM->SBUF eviction (DVE multiply). fp32 accumulation in
    PSUM throughout.
  - Host: combine — scatter-add the two expert contributions per token.
"""

import numpy as np
import ml_dtypes

import sys

if "/opt/trn_rl_repo" not in sys.path:
    sys.path.insert(0, "/opt/trn_rl_repo")

import concourse.tile as tile
from concourse import bacc, mybir
from concourse.bass_utils import run_bass_kernel_spmd

BF16 = ml_dtypes.bfloat16

E, D, F, NTOK = 8, 1024, 4096, 2048
P = 128
KD, KF = D // P, F // P  # 8, 32

_NC_CACHE: dict = {}


def _plan_capacity(max_count: int):
    """Pick capacity C = n_chunks * L with L <= 512, L a multiple of 16."""
    max_count = max(max_count, 16)
    n = -(-max_count // 512)  # ceil
    L = -(-max_count // (n * 16)) * 16
    return n * L, L, n


def _build_nc(C: int, L: int):
    nchunks = C // L
    # Bacc (not raw Bass): its finalize() runs move_matmul_waits_to_ldweights
    # + generate_event_semaphores, which split multi-wait instructions down to
    # the TRN2 limit of 1 sync wait per instruction.
    nc = bacc.Bacc(None)
    # x packed per chunk:   x_pack[c, p, k, t]  = x_e.T[k*P + p, c*L + t]
    # w1 packed per f-tile: w1_pack[f, p, k, c] = w1[e][f*P + c, k*P + p]
    # w2 packed per d-tile: w2_pack[d, p, k2, c] = w2[e][d*P + c, k2*P + p]
    x_in = nc.declare_dram_parameter("x_pack", [nchunks, P, KD, L], mybir.dt.bfloat16, isOutput=False)
    w1_in = nc.declare_dram_parameter("w1_pack", [KF, P, KD, P], mybir.dt.bfloat16, isOutput=False)
    w2_in = nc.declare_dram_parameter("w2_pack", [KD, P, KF, P], mybir.dt.bfloat16, isOutput=False)
    cf_in = nc.declare_dram_parameter("coef", [P, C], mybir.dt.float32, isOutput=False)
    y_out = nc.declare_dram_parameter("y_dc", [KD, P, C], mybir.dt.float32, isOutput=True)

    with tile.TileContext(nc) as tc:
        with (
            tc.tile_pool(name="wpool", bufs=1) as wpool,
            tc.tile_pool(name="apool", bufs=1) as apool,
            tc.tile_pool(name="ps1", bufs=4, space="PSUM") as ps1,
            tc.tile_pool(name="ps2", bufs=4, space="PSUM") as ps2,
            tc.tile_pool(name="ypool", bufs=6) as ypool,
        ):
            # ---- loads, in consumption order, one DMA per consumer group.
            # w1_0 first, then x chunks, then remaining w1: the first S1 group
            # needs w1_0 + x_0 and everything lands on one HWDGE queue in
            # program order.
            w1_sb = [None] * KF
            x_sb = [None] * nchunks

            def load_w1(f):
                wt = wpool.tile([P, KD, P], mybir.dt.bfloat16, name=f"w1_{f}")
                nc.sync.dma_start(wt, w1_in[f])
                w1_sb[f] = wt

            load_w1(0)
            for c in range(nchunks):
                xt = apool.tile([P, KD, L], mybir.dt.bfloat16, name=f"x_{c}")
                nc.sync.dma_start(xt, x_in[c])
                x_sb[c] = xt
            for f in range(1, KF):
                load_w1(f)
            coef_sb = apool.tile([P, C], mybir.dt.float32, name="coef_sb")
            nc.sync.dma_start(coef_sb, cf_in[:])
            w2_sb = []
            for d in range(KD):
                wt = wpool.tile([P, KF, P], mybir.dt.bfloat16, name=f"w2_{d}")
                nc.sync.dma_start(wt, w2_in[d])
                w2_sb.append(wt)

            h_sb = [
                apool.tile([P, C], mybir.dt.bfloat16, name=f"h_{k2}") for k2 in range(KF)
            ]

            # ---- stage 1: h.T[f*P:(f+1)*P, :] = gelu(w1 @ x.T) per F-tile
            for f in range(KF):
                for c in range(nchunks):
                    c0 = c * L
                    ps = ps1.tile([P, L], mybir.dt.float32, name="ps1t", tag="ps1t")
                    for k in range(KD):
                        nc.tensor.matmul(
                            ps,
                            w1_sb[f][:, k],
                            x_sb[c][:, k],
                            start=(k == 0),
                            stop=(k == KD - 1),
                        )
                    nc.scalar.activation(
                        out=h_sb[f][:, c0 : c0 + L],
                        in_=ps,
                        func=mybir.ActivationFunctionType.Gelu,
                    )

            # ---- stage 2: y.T[d*P:(d+1)*P, :] = (w2 @ h.T) * coef
            for d in range(KD):
                for c in range(nchunks):
                    c0 = c * L
                    ps = ps2.tile([P, L], mybir.dt.float32, name="ps2t", tag="ps2t")
                    for k2 in range(KF):
                        nc.tensor.matmul(
                            ps,
                            w2_sb[d][:, k2],
                            h_sb[k2][:, c0 : c0 + L],
                            start=(k2 == 0),
                            stop=(k2 == KF - 1),
                        )
                    y_sb = ypool.tile([P, L], mybir.dt.float32, name="y_sb", tag="y_sb")
                    nc.vector.tensor_mul(y_sb, ps, coef_sb[:, c0 : c0 + L])
                    nc.sync.dma_start(y_out[d][:, c0 : c0 + L], y_sb)
    nc.finalize()
    return nc


def _route(x: np.ndarray, gate_w: np.ndarray):
    """fp64 gating; matches reference fp32 routing (selection gaps >> fp32 eps)."""
    logits = x.astype(np.float64) @ gate_w.astype(np.float64).T  # [N, E]
    top2 = np.argsort(-logits, axis=1, kind="stable")[:, :2]  # [N, 2]
    v = np.take_along_axis(logits, top2, axis=1)
    v = v - v.max(axis=1, keepdims=True)
    ew = np.exp(v)
    w = ew / ew.sum(axis=1, keepdims=True)  # [N, 2]
    return top2, w.astype(np.float32)


def _run(inputs: dict, trace: bool = False, trace_cores=None):
    x = np.asarray(inputs["x"], dtype=np.float32)
    gate_w = np.asarray(inputs["gate_w"], dtype=np.float32)
    w1 = np.asarray(inputs["w1"], dtype=np.float32)
    w2 = np.asarray(inputs["w2"], dtype=np.float32)
    n = x.shape[0]

    top2, wsm = _route(x, gate_w)

    idx_list, coef_list = [], []
    for e in range(E):
        mask = top2 == e  # [N, 2]
        sel = mask.any(axis=1)
        idx = np.nonzero(sel)[0]
        we = np.where(mask[idx, 0], wsm[idx, 0], wsm[idx, 1])
        idx_list.append(idx)
        coef_list.append(we.astype(np.float32))

    max_count = max(len(i) for i in idx_list)
    C, L, nchunks = _plan_capacity(max_count)

    key = (C, L)
    if key not in _NC_CACHE:
        _NC_CACHE[key] = _build_nc(C, L)
    nc = _NC_CACHE[key]

    in_maps = []
    for e in range(E):
        idx, cf = idx_list[e], coef_list[e]
        cnt = len(idx)
        xe = np.zeros((D, C), dtype=BF16)
        xe[:, :cnt] = x[idx].T.astype(BF16)
        # [D, C] -> [k, p, c, t] -> pack [c, p, k, t]
        x_pack = np.ascontiguousarray(
            xe.reshape(KD, P, nchunks, L).transpose(2, 1, 0, 3)
        )
        coef = np.zeros((C,), dtype=np.float32)
        coef[:cnt] = cf
        coef_rep = np.ascontiguousarray(np.broadcast_to(coef, (P, C)))
        # w1[e] is [F, D]: [f, c, k, p] -> pack [f, p, k, c]
        w1_pack = np.ascontiguousarray(
            w1[e].astype(BF16).reshape(KF, P, KD, P).transpose(0, 3, 2, 1)
        )
        # w2[e] is [D, F]: [d, c, k2, p] -> pack [d, p, k2, c]
        w2_pack = np.ascontiguousarray(
            w2[e].astype(BF16).reshape(KD, P, KF, P).transpose(0, 3, 2, 1)
        )
        in_maps.append(
            {
                "x_pack": x_pack,
                "w1_pack": w1_pack,
                "w2_pack": w2_pack,
                "coef": coef_rep,
            }
        )

    res = run_bass_kernel_spmd(
        nc,
        in_maps,
        list(range(E)),
        trace=trace,
        trace_cores=trace_cores,
    )

    out = np.zeros((n, D), dtype=np.float32)
    for e in range(E):
        idx = idx_list[e]
        cnt = len(idx)
        y_dc = np.asarray(res.results[e]["y_dc"], dtype=np.float32)  # [KD, P, C]
        y = y_dc.reshape(D, C)[:, :cnt]  # [D, cnt]
        out[idx] += y.T
    return out, res


def kernel(**inputs) -> np.ndarray:
    out, _ = _run(inputs, trace=False)
    return out


if __name__ == "__main__":
    rng = np.random.default_rng(0)
    fake = {
        "x": rng.standard_normal((NTOK, D), dtype=np.float32),
        "gate_w": (rng.standard_normal((E, D)) * 0.02).astype(np.float32),
        "w1": (rng.standard_normal((E, F, D)) * 0.02).astype(np.float32),
        "w2": (rng.standard_normal((E, D, F)) * 0.02).astype(np.float32),
    }
    out = kernel(**fake)
    print("ok", out.shape, out.dtype, np.abs(out).max())


# revision 11
# speedup vs baseline: 1.0107x; 1.0107x over previous
"""MoE (top-2 routing, E=8 experts) Trainium2 kernel.

Strategy (expert-parallel across 8 NeuronCores):
  - Host: gate (tiny: [2048,1024]@[1024,8]) in fp64, top-2 + softmax exactly
    reproducing the reference routing (selection gaps are ~4e-4, far above
    fp32 noise, so fp64 routing == reference fp32 routing).
  - Host: dispatch — gather each expert's tokens into a fixed-capacity,
    zero-padded buffer; one expert per core. Weights are pre-transposed,
    cast to bf16 and packed per matmul-group on host so each core's full
    working set (w1.T 8MiB + w2.T 8MiB bf16) is SBUF-resident and every
    DMA is contiguous per partition and arrives in consumption order.
  - Device (per core): h.T = gelu(w1 @ x.T) [F x C], y.T = w2 @ h.T [D x C],
    tokens live in the matmul free dimension. Routing coefficient is applied
    as part of the PSUM->SBUF eviction (DVE multiply). fp32 accumulation in
    PSUM throughout.
  - Host: combine — scatter-add the two expert contributions per token.
"""

import numpy as np
import ml_dtypes

import sys

if "/opt/trn_rl_repo" not in sys.path:
    sys.path.insert(0, "/opt/trn_rl_repo")

import concourse.tile as tile
from concourse import bacc, mybir
from concourse.bass_utils import run_bass_kernel_spmd

BF16 = ml_dtypes.bfloat16

E, D, F, NTOK = 8, 1024, 4096, 2048
P = 128
KD, KF = D // P, F // P  # 8, 32

_NC_CACHE: dict = {}


def _plan_capacity(max_count: int):
    """Pick capacity C = n_chunks * L with L <= 512, L a multiple of 16."""
    max_count = max(max_count, 16)
    n = -(-max_count // 512)  # ceil
    L = -(-max_count // (n * 16)) * 16
    return n * L, L, n


def _build_nc(C: int, L: int):
    nchunks = C // L
    # Bacc (not raw Bass): its finalize() runs move_matmul_waits_to_ldweights
    # + generate_event_semaphores, which split multi-wait instructions down to
    # the TRN2 limit of 1 sync wait per instruction.
    nc = bacc.Bacc(None)
    # x packed per chunk:   x_pack[c, p, k, t]  = x_e.T[k*P + p, c*L + t]
    # w1 packed per f-tile: w1_pack[f, p, k, c] = w1[e][f*P + c, k*P + p]
    # w2 packed per d-tile: w2_pack[d, p, k2, c] = w2[e][d*P + c, k2*P + p]
    x_in = nc.declare_dram_parameter("x_pack", [nchunks, P, KD, L], mybir.dt.bfloat16, isOutput=False)
    w1_in = nc.declare_dram_parameter("w1_pack", [KF, P, KD, P], mybir.dt.bfloat16, isOutput=False)
    w2_in = nc.declare_dram_parameter("w2_pack", [KD, P, KF, P], mybir.dt.bfloat16, isOutput=False)
    cf_in = nc.declare_dram_parameter("coef", [P, C], mybir.dt.float32, isOutput=False)
    y_out = nc.declare_dram_parameter("y_dc", [KD, P, C], mybir.dt.float32, isOutput=True)

    with tile.TileContext(nc) as tc:
        with (
            tc.tile_pool(name="wpool", bufs=1) as wpool,
            tc.tile_pool(name="apool", bufs=1) as apool,
            tc.tile_pool(name="ps1", bufs=4, space="PSUM") as ps1,
            tc.tile_pool(name="ps2", bufs=3, space="PSUM") as ps2,
            tc.tile_pool(name="psw", bufs=1, space="PSUM") as psw,
            tc.tile_pool(name="ypool", bufs=6) as ypool,
        ):
            # ---- PE warm-up: ~4us of dependency-free matmuls run during the
            # input-DMA head so the HAM clock gate is at 8/8 (2.4 GHz) when
            # the real stream starts (saves ~3us of cold-clock matmuls).
            warm = wpool.tile([P, P], mybir.dt.bfloat16, name="warm")
            nc.vector.memset(warm, 0.0)
            ps_w = psw.tile([P, 64], mybir.dt.float32, name="ps_w")
            for i in range(64):
                nc.tensor.matmul(
                    ps_w, warm, warm[:, :64], start=(i == 0), stop=(i == 63)
                )
            # ---- loads, in consumption order, one DMA per consumer group.
            # w1_0 first, then x chunks, then remaining w1: the first S1 group
            # needs w1_0 + x_0 and everything lands on one HWDGE queue in
            # program order.
            w1_sb = [None] * KF
            x_sb = [None] * nchunks

            def load_w1(f):
                wt = wpool.tile([P, KD, P], mybir.dt.bfloat16, name=f"w1_{f}")
                nc.sync.dma_start(wt, w1_in[f])
                w1_sb[f] = wt

            load_w1(0)
            for c in range(nchunks):
                xt = apool.tile([P, KD, L], mybir.dt.bfloat16, name=f"x_{c}")
                nc.sync.dma_start(xt, x_in[c])
                x_sb[c] = xt
            for f in range(1, KF):
                load_w1(f)
            coef_sb = apool.tile([P, C], mybir.dt.float32, name="coef_sb")
            nc.sync.dma_start(coef_sb, cf_in[:])
            w2_sb = []
            for d in range(KD):
                wt = wpool.tile([P, KF, P], mybir.dt.bfloat16, name=f"w2_{d}")
                nc.sync.dma_start(wt, w2_in[d])
                w2_sb.append(wt)

            h_sb = [
                apool.tile([P, C], mybir.dt.bfloat16, name=f"h_{k2}") for k2 in range(KF)
            ]

            # ---- stage 1: h.T[f*P:(f+1)*P, :] = gelu(w1 @ x.T) per F-tile
            for f in range(KF):
                for c in range(nchunks):
                    c0 = c * L
                    ps = ps1.tile([P, L], mybir.dt.float32, name="ps1t", tag="ps1t")
                    for k in range(KD):
                        nc.tensor.matmul(
                            ps,
                            w1_sb[f][:, k],
                            x_sb[c][:, k],
                            start=(k == 0),
                            stop=(k == KD - 1),
                        )
                    nc.scalar.activation(
                        out=h_sb[f][:, c0 : c0 + L],
                        in_=ps,
                        func=mybir.ActivationFunctionType.Gelu,
                    )

            # ---- stage 2: y.T[d*P:(d+1)*P, :] = (w2 @ h.T) * coef
            for d in range(KD):
                for c in range(nchunks):
                    c0 = c * L
                    ps = ps2.tile([P, L], mybir.dt.float32, name="ps2t", tag="ps2t")
                    for k2 in range(KF):
                        nc.tensor.matmul(
                            ps,
                            w2_sb[d][:, k2],
                            h_sb[k2][:, c0 : c0 + L],
                            start=(k2 == 0),
                            stop=(k2 == KF - 1),
                        )
                    y_sb = ypool.tile([P, L], mybir.dt.float32, name="y_sb", tag="y_sb")
                    nc.vector.tensor_mul(y_sb, ps, coef_sb[:, c0 : c0 + L])
                    nc.sync.dma_start(y_out[d][:, c0 : c0 + L], y_sb)
    nc.finalize()
    return nc


def _route(x: np.ndarray, gate_w: np.ndarray):
    """fp64 gating; matches reference fp32 routing (selection gaps >> fp32 eps)."""
    logits = x.astype(np.float64) @ gate_w.astype(np.float64).T  # [N, E]
    top2 = np.argsort(-logits, axis=1, kind="stable")[:, :2]  # [N, 2]
    v = np.take_along_axis(logits, top2, axis=1)
    v = v - v.max(axis=1, keepdims=True)
    ew = np.exp(v)
    w = ew / ew.sum(axis=1, keepdims=True)  # [N, 2]
    return top2, w.astype(np.float32)


def _run(inputs: dict, trace: bool = False, trace_cores=None):
    x = np.asarray(inputs["x"], dtype=np.float32)
    gate_w = np.asarray(inputs["gate_w"], dtype=np.float32)
    w1 = np.asarray(inputs["w1"], dtype=np.float32)
    w2 = np.asarray(inputs["w2"], dtype=np.float32)
    n = x.shape[0]

    top2, wsm = _route(x, gate_w)

    idx_list, coef_list = [], []
    for e in range(E):
        mask = top2 == e  # [N, 2]
        sel = mask.any(axis=1)
        idx = np.nonzero(sel)[0]
        we = np.where(mask[idx, 0], wsm[idx, 0], wsm[idx, 1])
        idx_list.append(idx)
        coef_list.append(we.astype(np.float32))

    max_count = max(len(i) for i in idx_list)
    C, L, nchunks = _plan_capacity(max_count)

    key = (C, L)
    if key not in _NC_CACHE:
        _NC_CACHE[key] = _build_nc(C, L)
    nc = _NC_CACHE[key]

    in_maps = []
    for e in range(E):
        idx, cf = idx_list[e], coef_list[e]
        cnt = len(idx)
        xe = np.zeros((D, C), dtype=BF16)
        xe[:, :cnt] = x[idx].T.astype(BF16)
        # [D, C] -> [k, p, c, t] -> pack [c, p, k, t]
        x_pack = np.ascontiguousarray(
            xe.reshape(KD, P, nchunks, L).transpose(2, 1, 0, 3)
        )
        coef = np.zeros((C,), dtype=np.float32)
        coef[:cnt] = cf
        coef_rep = np.ascontiguousarray(np.broadcast_to(coef, (P, C)))
        # w1[e] is [F, D]: [f, c, k, p] -> pack [f, p, k, c]
        w1_pack = np.ascontiguousarray(
            w1[e].astype(BF16).reshape(KF, P, KD, P).transpose(0, 3, 2, 1)
        )
        # w2[e] is [D, F]: [d, c, k2, p] -> pack [d, p, k2, c]
        w2_pack = np.ascontiguousarray(
            w2[e].astype(BF16).reshape(KD, P, KF, P).transpose(0, 3, 2, 1)
        )
        in_maps.append(
            {
                "x_pack": x_pack,
                "w1_pack": w1_pack,
                "w2_pack": w2_pack,
                "coef": coef_rep,
            }
        )

    res = run_bass_kernel_spmd(
        nc,
        in_maps,
        list(range(E)),
        trace=trace,
        trace_cores=trace_cores,
    )

    out = np.zeros((n, D), dtype=np.float32)
    for e in range(E):
        idx = idx_list[e]
        cnt = len(idx)
        y_dc = np.asarray(res.results[e]["y_dc"], dtype=np.float32)  # [KD, P, C]
        y = y_dc.reshape(D, C)[:, :cnt]  # [D, cnt]
        out[idx] += y.T
    return out, res


def kernel(**inputs) -> np.ndarray:
    out, _ = _run(inputs, trace=False)
    return out


if __name__ == "__main__":
    rng = np.random.default_rng(0)
    fake = {
        "x": rng.standard_normal((NTOK, D), dtype=np.float32),
        "gate_w": (rng.standard_normal((E, D)) * 0.02).astype(np.float32),
        "w1": (rng.standard_normal((E, F, D)) * 0.02).astype(np.float32),
        "w2": (rng.standard_normal((E, D, F)) * 0.02).astype(np.float32),
    }
    out = kernel(**fake)
    print("ok", out.shape, out.dtype, np.abs(out).max())


# revision 12
# speedup vs baseline: 1.0209x; 1.0101x over previous
"""MoE (top-2 routing, E=8 experts) Trainium2 kernel.

Strategy (expert-parallel across 8 NeuronCores):
  - Host: gate (tiny: [2048,1024]@[1024,8]) in fp64, top-2 + softmax exactly
    reproducing the reference routing (selection gaps are ~4e-4, far above
    fp32 noise, so fp64 routing == reference fp32 routing).
  - Host: dispatch — gather each expert's tokens into a fixed-capacity,
    zero-padded buffer; one expert per core. Weights are pre-transposed,
    cast to bf16 and packed per matmul-group on host so each core's full
    working set (w1.T 8MiB + w2.T 8MiB bf16) is SBUF-resident and every
    DMA is contiguous per partition and arrives in consumption order.
  - Device (per core): h.T = gelu(w1 @ x.T) [F x C], y.T = w2 @ h.T [D x C],
    tokens live in the matmul free dimension. Routing coefficient is applied
    as part of the PSUM->SBUF eviction (DVE multiply). fp32 accumulation in
    PSUM throughout.
  - Host: combine — scatter-add the two expert contributions per token.
"""

import numpy as np
import ml_dtypes

import sys

if "/opt/trn_rl_repo" not in sys.path:
    sys.path.insert(0, "/opt/trn_rl_repo")

import concourse.tile as tile
from concourse import bacc, mybir
from concourse.bass_utils import run_bass_kernel_spmd

BF16 = ml_dtypes.bfloat16

E, D, F, NTOK = 8, 1024, 4096, 2048
P = 128
KD, KF = D // P, F // P  # 8, 32

_NC_CACHE: dict = {}


def _plan_capacity(max_count: int):
    """Pick capacity C = n_chunks * L with L <= 512, L a multiple of 16."""
    max_count = max(max_count, 16)
    n = -(-max_count // 512)  # ceil
    L = -(-max_count // (n * 16)) * 16
    return n * L, L, n


def _build_nc(C: int, L: int):
    nchunks = C // L
    # Bacc (not raw Bass): its finalize() runs move_matmul_waits_to_ldweights
    # + generate_event_semaphores, which split multi-wait instructions down to
    # the TRN2 limit of 1 sync wait per instruction.
    nc = bacc.Bacc(None)
    # x packed per chunk:   x_pack[c, p, k, t]  = x_e.T[k*P + p, c*L + t]
    # w1 packed per f-tile: w1_pack[f, p, k, c] = w1[e][f*P + c, k*P + p]
    # w2 packed per d-tile: w2_pack[d, p, k2, c] = w2[e][d*P + c, k2*P + p]
    x_in = nc.declare_dram_parameter("x_pack", [nchunks, P, KD, L], mybir.dt.bfloat16, isOutput=False)
    w1_in = nc.declare_dram_parameter("w1_pack", [KF, P, KD, P], mybir.dt.bfloat16, isOutput=False)
    w2_in = nc.declare_dram_parameter("w2_pack", [KD, P, KF, P], mybir.dt.bfloat16, isOutput=False)
    cf_in = nc.declare_dram_parameter("coef", [P, C], mybir.dt.float32, isOutput=False)
    y_out = nc.declare_dram_parameter("y_dc", [KD, P, C], mybir.dt.float32, isOutput=True)

    with tile.TileContext(nc) as tc:
        with (
            tc.tile_pool(name="wpool", bufs=1) as wpool,
            tc.tile_pool(name="apool", bufs=1) as apool,
            tc.tile_pool(name="ps1", bufs=4, space="PSUM") as ps1,
            tc.tile_pool(name="ps2", bufs=3, space="PSUM") as ps2,
            tc.tile_pool(name="psw", bufs=1, space="PSUM") as psw,
            tc.tile_pool(name="ypool", bufs=6) as ypool,
        ):
            # ---- PE warm-up: ~4us of dependency-free matmuls run during the
            # input-DMA head so the HAM clock gate is at 8/8 (2.4 GHz) when
            # the real stream starts (saves ~3us of cold-clock matmuls).
            warm = wpool.tile([P, P], mybir.dt.bfloat16, name="warm")
            nc.vector.memset(warm, 0.0)
            ps_w = psw.tile([P, 64], mybir.dt.float32, name="ps_w")
            for i in range(112):
                nc.tensor.matmul(
                    ps_w, warm, warm[:, :64], start=(i == 0), stop=(i == 111)
                )
            # ---- loads, in consumption order, one DMA per consumer group.
            # w1_0 first, then x chunks, then remaining w1: the first S1 group
            # needs w1_0 + x_0 and everything lands on one HWDGE queue in
            # program order.
            w1_sb = [None] * KF
            x_sb = [None] * nchunks

            def load_w1(f):
                wt = wpool.tile([P, KD, P], mybir.dt.bfloat16, name=f"w1_{f}")
                nc.sync.dma_start(wt, w1_in[f])
                w1_sb[f] = wt

            load_w1(0)
            for c in range(nchunks):
                xt = apool.tile([P, KD, L], mybir.dt.bfloat16, name=f"x_{c}")
                nc.sync.dma_start(xt, x_in[c])
                x_sb[c] = xt
            for f in range(1, KF):
                load_w1(f)
            coef_sb = apool.tile([P, C], mybir.dt.float32, name="coef_sb")
            nc.sync.dma_start(coef_sb, cf_in[:])
            w2_sb = []
            for d in range(KD):
                wt = wpool.tile([P, KF, P], mybir.dt.bfloat16, name=f"w2_{d}")
                nc.sync.dma_start(wt, w2_in[d])
                w2_sb.append(wt)

            h_sb = [
                apool.tile([P, C], mybir.dt.bfloat16, name=f"h_{k2}") for k2 in range(KF)
            ]

            # ---- stage 1: h.T[f*P:(f+1)*P, :] = gelu(w1 @ x.T) per F-tile
            for f in range(KF):
                for c in range(nchunks):
                    c0 = c * L
                    ps = ps1.tile([P, L], mybir.dt.float32, name="ps1t", tag="ps1t")
                    for k in range(KD):
                        nc.tensor.matmul(
                            ps,
                            w1_sb[f][:, k],
                            x_sb[c][:, k],
                            start=(k == 0),
                            stop=(k == KD - 1),
                        )
                    nc.scalar.activation(
                        out=h_sb[f][:, c0 : c0 + L],
                        in_=ps,
                        func=mybir.ActivationFunctionType.Gelu,
                    )

            # ---- stage 2: y.T[d*P:(d+1)*P, :] = (w2 @ h.T) * coef
            for d in range(KD):
                for c in range(nchunks):
                    c0 = c * L
                    ps = ps2.tile([P, L], mybir.dt.float32, name="ps2t", tag="ps2t")
                    for k2 in range(KF):
                        nc.tensor.matmul(
                            ps,
                            w2_sb[d][:, k2],
                            h_sb[k2][:, c0 : c0 + L],
                            start=(k2 == 0),
                            stop=(k2 == KF - 1),
                        )
                    y_sb = ypool.tile([P, L], mybir.dt.float32, name="y_sb", tag="y_sb")
                    nc.vector.tensor_mul(y_sb, ps, coef_sb[:, c0 : c0 + L])
                    nc.sync.dma_start(y_out[d][:, c0 : c0 + L], y_sb)
    nc.finalize()
    return nc


def _route(x: np.ndarray, gate_w: np.ndarray):
    """fp64 gating; matches reference fp32 routing (selection gaps >> fp32 eps)."""
    logits = x.astype(np.float64) @ gate_w.astype(np.float64).T  # [N, E]
    top2 = np.argsort(-logits, axis=1, kind="stable")[:, :2]  # [N, 2]
    v = np.take_along_axis(logits, top2, axis=1)
    v = v - v.max(axis=1, keepdims=True)
    ew = np.exp(v)
    w = ew / ew.sum(axis=1, keepdims=True)  # [N, 2]
    return top2, w.astype(np.float32)


def _run(inputs: dict, trace: bool = False, trace_cores=None):
    x = np.asarray(inputs["x"], dtype=np.float32)
    gate_w = np.asarray(inputs["gate_w"], dtype=np.float32)
    w1 = np.asarray(inputs["w1"], dtype=np.float32)
    w2 = np.asarray(inputs["w2"], dtype=np.float32)
    n = x.shape[0]

    top2, wsm = _route(x, gate_w)

    idx_list, coef_list = [], []
    for e in range(E):
        mask = top2 == e  # [N, 2]
        sel = mask.any(axis=1)
        idx = np.nonzero(sel)[0]
        we = np.where(mask[idx, 0], wsm[idx, 0], wsm[idx, 1])
        idx_list.append(idx)
        coef_list.append(we.astype(np.float32))

    max_count = max(len(i) for i in idx_list)
    C, L, nchunks = _plan_capacity(max_count)

    key = (C, L)
    if key not in _NC_CACHE:
        _NC_CACHE[key] = _build_nc(C, L)
    nc = _NC_CACHE[key]

    in_maps = []
    for e in range(E):
        idx, cf = idx_list[e], coef_list[e]
        cnt = len(idx)
        xe = np.zeros((D, C), dtype=BF16)
        xe[:, :cnt] = x[idx].T.astype(BF16)
        # [D, C] -> [k, p, c, t] -> pack [c, p, k, t]
        x_pack = np.ascontiguousarray(
            xe.reshape(KD, P, nchunks, L).transpose(2, 1, 0, 3)
        )
        coef = np.zeros((C,), dtype=np.float32)
        coef[:cnt] = cf
        coef_rep = np.ascontiguousarray(np.broadcast_to(coef, (P, C)))
        # w1[e] is [F, D]: [f, c, k, p] -> pack [f, p, k, c]
        w1_pack = np.ascontiguousarray(
            w1[e].astype(BF16).reshape(KF, P, KD, P).transpose(0, 3, 2, 1)
        )
        # w2[e] is [D, F]: [d, c, k2, p] -> pack [d, p, k2, c]
        w2_pack = np.ascontiguousarray(
            w2[e].astype(BF16).reshape(KD, P, KF, P).transpose(0, 3, 2, 1)
        )
        in_maps.append(
            {
                "x_pack": x_pack,
                "w1_pack": w1_pack,
                "w2_pack": w2_pack,
                "coef": coef_rep,
            }
        )

    res = run_bass_kernel_spmd(
        nc,
        in_maps,
        list(range(E)),
        trace=trace,
        trace_cores=trace_cores,
    )

    out = np.zeros((n, D), dtype=np.float32)
    for e in range(E):
        idx = idx_list[e]
        cnt = len(idx)
        y_dc = np.asarray(res.results[e]["y_dc"], dtype=np.float32)  # [KD, P, C]
        y = y_dc.reshape(D, C)[:, :cnt]  # [D, cnt]
        out[idx] += y.T
    return out, res


def kernel(**inputs) -> np.ndarray:
    out, _ = _run(inputs, trace=False)
    return out


if __name__ == "__main__":
    rng = np.random.default_rng(0)
    fake = {
        "x": rng.standard_normal((NTOK, D), dtype=np.float32),
        "gate_w": (rng.standard_normal((E, D)) * 0.02).astype(np.float32),
        "w1": (rng.standard_normal((E, F, D)) * 0.02).astype(np.float32),
        "w2": (rng.standard_normal((E, D, F)) * 0.02).astype(np.float32),
    }
    out = kernel(**fake)
    print("ok", out.shape, out.dtype, np.abs(out).max())
